# revision 2
# baseline (speedup 1.0000x reference)
"""Local (windowed) self-attention Trainium2 kernel, v2.

Model (reference): LayerNorm -> per-window (W=1024) multi-head attention
(H=8 heads, K=32 head dim) -> output projection -> residual add.
Shapes: x [B=2, T=8192, C=512]; 16 independent windows of 1024 tokens.
16 windows / 8 cores = 2 windows per core, weights replicated, no
collectives.

v2 restructure, driven by real-HW NTFF profiles (not the cost model):
  - LN z-write on DVE (644ns/[128,512]) -- Pool runs the same op at 8.3us.
  - zT via PE identity-matmul transposes + int32-bitcast PSUM->SBUF copies;
    no DRAM bounce (the v1 bounce kept a DMA queue ~90% busy).
  - Softmax denominators: packed AV layout [den|num|num|den] so normalize
    is 2x reciprocal_approx_fast [32,512] (802ns vs 4006ns InstReciprocal)
    + ONE [64,512] mul per head pair.
  - Scores: 4-way tile_position row packing (K=32 per head) -- 4 heads
    co-stream in ~one MM duration (measured, the v1 "no concurrency"
    conclusion was cost-model-only).
  - Output projection: fp8e4 DoubleRow (oT written fp8 by the normalize
    mul, x16 scale folded into the AV ones-columns and Wo; 1/256 unscale
    folded into the fused affine_then_add residual).
  - Exp: ScalarE activation, with a tunable fraction offloaded to DVE via
    Schraudolph int16 bit-trick (1442ns vs 1336ns per [128,1024] -- the
    engines run in parallel).
  - PE warm-up dummy MMs in the LN head to lift the HAM clock gate
    (PE runs 1.2GHz until ~3.4us of sustained activity, 2.4GHz after).
"""

import numpy as np
import ml_dtypes

import concourse.bass as bass
import concourse.tile as tile
from concourse import bacc, mybir
from concourse.bass_utils import run_bass_kernel_spmd

F32 = mybir.dt.float32
BF16 = mybir.dt.bfloat16
FP8 = mybir.dt.float8e4
I32 = mybir.dt.int32
I16 = mybir.dt.int16

B, T, C, H, K = 2, 8192, 512, 8, 32
W = 1024
HK = H * K              # 256
N_CORES = 8
NW = (B * T) // W       # 16 windows
WPC = NW // N_CORES     # 2 windows per core
EPS = 1e-5
SCALE = 1.0 / np.sqrt(K)

TOK_TILES = W // 128    # 8 token tiles per window
C_CHUNKS = C // 128     # 4
Q_TILES = W // 512      # 2 query tiles per window
S_CHUNKS = W // 128     # 8 key chunks per window
M_GROUPS = 2            # head groups of 4 (heads 4m..4m+3)
OSCALE = 16.0           # fp8 dynamic-range scale for oT / Wo
WSCALE = 16.0           # fp8 dynamic-range scale for QKV weights
N_STEPS = Q_TILES * M_GROUPS * S_CHUNKS  # 32 attention steps per window


def _build_program(reps=1, has_bo=False, warm_mms=100, dve_exp_mod=3,
                   fp8_out=True, do_attn=True, debug_outs=False,
                   recip_mode="copy_fast", qkv_dr=False):
    nc = bacc.Bacc("TRN2", target_bir_lowering=False, debug=False)

    x_d = nc.dram_tensor("x", [WPC * W, C], F32, kind="ExternalInput")
    wqkv_d = nc.dram_tensor(
        "wqkv", [3, C_CHUNKS, 128, HK], FP8 if qkv_dr else BF16,
        kind="ExternalInput"
    )
    if fp8_out:
        wo_d = nc.dram_tensor("wo", [128, 2, C], FP8, kind="ExternalInput")
    else:
        wo_d = nc.dram_tensor("wo", [2, 128, C], BF16, kind="ExternalInput")
    bqk_d = nc.dram_tensor("bqk", [128, 4], F32, kind="ExternalInput")
    ident_d = nc.dram_tensor("ident", [128, 128], BF16, kind="ExternalInput")
    bo_d = (
        nc.dram_tensor("bo", [1, C], F32, kind="ExternalInput")
        if has_bo else None
    )
    out_d = nc.dram_tensor("out", [WPC * W, C], F32, kind="ExternalOutput")
    if debug_outs:
        dbg_zt_d = nc.dram_tensor("dbg_zt", [128, C_CHUNKS * W], BF16,
                                  kind="ExternalOutput")
        dbg_q_d = nc.dram_tensor("dbg_q", [128, W], BF16,
                                 kind="ExternalOutput")
        dbg_k_d = nc.dram_tensor("dbg_k", [128, W], BF16,
                                 kind="ExternalOutput")
        dbg_v_d = nc.dram_tensor("dbg_v", [128, S_CHUNKS * 4 * 2 * 2 * 32],
                                 BF16, kind="ExternalOutput")
        dbg_ot_d = nc.dram_tensor("dbg_ot", [128, 2 * W], F32,
                                  kind="ExternalOutput")

    with tile.TileContext(nc) as tc:
        with (
            tc.tile_pool(name="const", bufs=1) as const,
            tc.tile_pool(name="xres", bufs=1) as xres,
            tc.tile_pool(name="zt", bufs=1) as ztp,
            tc.tile_pool(name="ln", bufs=6) as ln,
            tc.tile_pool(name="zw", bufs=4) as zw,
            tc.tile_pool(name="qk", bufs=2) as qk,
            tc.tile_pool(name="vp", bufs=2) as vp,
            tc.tile_pool(name="ot", bufs=2) as otp,
            tc.tile_pool(name="ex", bufs=6) as ex,
            tc.tile_pool(name="tmp", bufs=6) as tmp,
            tc.tile_pool(name="outp", bufs=6) as outp,
            tc.tile_pool(name="psA", bufs=1, space="PSUM") as psA,
            tc.tile_pool(name="psB", bufs=1, space="PSUM") as psB,
            tc.tile_pool(name="ps_acc", bufs=2, space="PSUM") as ps_acc,
            tc.tile_pool(name="ps_proj", bufs=2, space="PSUM") as ps_proj,
        ):
            from contextlib import ExitStack as _ES
            _es = _ES()
            if reps > 1:
                _es.enter_context(
                    tc.For_i(
                        0, reps, 1,
                        hint_engines=(
                            mybir.EngineType.PE,
                            mybir.EngineType.Activation,
                            mybir.EngineType.DVE,
                            mybir.EngineType.SP,
                        ),
                    )
                )

            # ---- constants / weights ---------------------------------
            w_all = const.tile([128, 3, C_CHUNKS, HK],
                               FP8 if qkv_dr else BF16)
            wq_s = w_all[:, 0]
            wk_s = w_all[:, 1]
            wv_s = w_all[:, 2]
            if fp8_out:
                wo_s = const.tile([128, 2, C], FP8)
            else:
                wo_s = const.tile([128, 2, C], BF16)
            bqk_s = const.tile([128, 4], F32)
            bq_s = bqk_s[:, 0:2]
            bk_s = bqk_s[:, 2:4]
            ident_s = const.tile([128, 128], BF16)
            ident8 = (const.tile([128, 128], FP8, name="ident8")
                      if qkv_dr else None)
            bo_s = const.tile([128, C], F32) if has_bo else None

            # score psum tiles: two [128,1024] (heads g0,g1 | g2,g3)
            scA = psA.tile([128, 1024], F32, name="scA")
            scB = psB.tile([128, 1024], F32, name="scB")

            def const_thunk():
                nc.sync.dma_start(ident_s, ident_d[0:][:128, :])
                if qkv_dr:
                    nc.vector.tensor_scalar(
                        out=ident8, in0=ident_s, scalar1=1.0, scalar2=0.0,
                        op0=mybir.AluOpType.mult, op1=mybir.AluOpType.add)
                nc.sync.dma_start(
                    w_all,
                    bass.AP(
                        tensor=wqkv_d.ap().tensor,
                        offset=0,
                        ap=[[HK, 128], [C_CHUNKS * 128 * HK, 3],
                            [128 * HK, C_CHUNKS], [1, HK]],
                    ),
                )
                if fp8_out:
                    nc.sync.dma_start(
                        wo_s,
                        bass.AP(
                            tensor=wo_d.ap().tensor,
                            offset=0,
                            ap=[[2 * C, 128], [C, 2], [1, C]],
                        ),
                    )
                else:
                    nc.sync.dma_start(
                        wo_s,
                        bass.AP(
                            tensor=wo_d.ap().tensor,
                            offset=0,
                            ap=[[C, 128], [128 * C, 2], [1, C]],
                        ),
                    )
                nc.sync.dma_start(bqk_s, bqk_d[0:][:128, :])
                if has_bo:
                    nc.sync.dma_start(
                        bo_s,
                        bass.AP(
                            tensor=bo_d.ap().tensor,
                            offset=0,
                            ap=[[0, 128], [1, C]],
                        ),
                    )

            # ---- per-window persistent tiles -------------------------
            xs = [
                [xres.tile([128, C], F32, name=f"x_{w}_{t}", tag=f"x_{w}_{t}")
                 for t in range(TOK_TILES)]
                for w in range(WPC)
            ]
            zT = [
                ztp.tile([128, C_CHUNKS, W], FP8 if qkv_dr else BF16,
                         name=f"zT_{w}", tag=f"zT_{w}")
                for w in range(WPC)
            ]
            qkt = {}
            for w in range(WPC):
                for name in ("q", "k"):
                    for m in range(M_GROUPS):
                        qkt[(w, name, m)] = qk.tile(
                            [128, W], BF16,
                            name=f"{name}T_{w}_{m}", tag=f"{name}T_{m}",
                        )
            # V packed per (s-chunk, head-pair u, half j): [V | ones/16]
            # -> acc rows per pair: [num_a | den_a | num_b | den_b]
            # (all 32-aligned base/span, the BIR partition-window rule)
            vs = {}
            vs_memsets = []
            for w in range(WPC):
                v_s = vp.tile([128, S_CHUNKS, 4, 2, 2, 32], BF16,
                              name=f"v_{w}", tag="v")
                vs[w] = v_s
                ones_val = (1.0 / OSCALE) if fp8_out else 1.0
                vs_memsets.append(
                    lambda v_s=v_s, ov=ones_val:
                        nc.gpsimd.memset(v_s[:, :, :, :, 1, :], ov)
                )
            oTs = {}
            for w in range(WPC):
                oTs[w] = otp.tile(
                    [128, 2, W], FP8 if fp8_out else BF16,
                    name=f"oT_{w}", tag="oT",
                )

            # ---- LayerNorm (stats on DVE, z-write DVE, zT via PE) ----
            def ln_thunks(w, split_dma=False):
                mvs = ln.tile([128, 2, TOK_TILES], F32, tag="mvs")
                rstds = ln.tile([128, TOK_TILES], F32, tag="rstds")

                def th_dma(t):
                    nc.sync.dma_start(
                        xs[w][t], x_d[(w * TOK_TILES + t) * 128:][:128, :]
                    )

                def th_a(t):
                    if not split_dma:
                        th_dma(t)
                    stats = ln.tile([128, 6], F32, tag="stats")
                    nc.vector.bn_stats(out=stats, in_=xs[w][t])
                    nc.vector.bn_aggr(out=mvs[:, :, t: t + 1], in_=stats)

                def th_b(lo, hi):
                    # rstd = 1/sqrt(var+eps): quake seed + 2 Newton steps
                    n = hi - lo
                    ve = ln.tile([128, n], F32, tag="ve")
                    nc.vector.tensor_scalar_add(out=ve, in0=mvs[:, 1, lo:hi],
                                                scalar1=float(EPS))
                    ti = ln.tile([128, n], I32, tag="ti")
                    nc.vector.tensor_scalar(
                        out=ti, in0=ve[:].bitcast(I32),
                        scalar1=1, scalar2=-1,
                        op0=mybir.AluOpType.logical_shift_right,
                        op1=mybir.AluOpType.bitwise_xor,
                    )
                    nc.vector.tensor_scalar_add(
                        out=ti, in0=ti, scalar1=0x5F3759DF + 1
                    )
                    y0 = ti[:].bitcast(F32)
                    t1 = ln.tile([128, n], F32, tag="t1")
                    t2 = ln.tile([128, n], F32, tag="t2")
                    nc.vector.tensor_mul(out=t1, in0=y0, in1=y0)
                    nc.vector.tensor_mul(out=t2, in0=t1, in1=ve)
                    nc.vector.tensor_scalar(
                        out=t1, in0=t2, scalar1=-0.5, scalar2=1.5,
                        op0=mybir.AluOpType.mult, op1=mybir.AluOpType.add,
                    )
                    nc.vector.tensor_mul(out=t2, in0=t1, in1=y0)  # y1
                    nc.vector.tensor_mul(out=t1, in0=t2, in1=t2)
                    nc.vector.tensor_mul(out=y0, in0=t1, in1=ve)
                    nc.vector.tensor_scalar(
                        out=t1, in0=y0, scalar1=-0.5, scalar2=1.5,
                        op0=mybir.AluOpType.mult, op1=mybir.AluOpType.add,
                    )
                    nc.vector.tensor_mul(out=rstds[:, lo:hi], in0=t1, in1=t2)

                def th_c(t):
                    # z = (x - mu) * rstd on DVE; PE transpose 4x [128,128]
                    # into one psum bank; copy into zT.
                    if qkv_dr:
                        # fp8 path: fp8 PE transpose writes element step 2
                        z_t = zw.tile([128, C], FP8, tag="z")
                        nc.vector.tensor_scalar(
                            out=z_t,
                            in0=xs[w][t],
                            scalar1=mvs[:, 0, t: t + 1],
                            scalar2=rstds[:, t: t + 1],
                            op0=mybir.AluOpType.subtract,
                            op1=mybir.AluOpType.mult,
                        )
                        tp = ps_proj.tile([128, C], BF16, name="ps_tp",
                                          tag="proj")
                        tp8 = tp[:].bitcast(FP8).rearrange(
                            "p (c n two) -> p c n two", c=C_CHUNKS, two=2)
                        for c in range(C_CHUNKS):
                            nc.tensor.transpose(
                                tp8[:, c, :, 0],
                                z_t[:, c * 128: (c + 1) * 128],
                                ident8,
                            )
                        nc.vector.tensor_copy(
                            zT[w][:, :, t * 128: (t + 1) * 128],
                            tp8[:, :, :, 0],
                        )
                        return
                    z_t = zw.tile([128, C], BF16, tag="z")
                    nc.vector.tensor_scalar(
                        out=z_t,
                        in0=xs[w][t],
                        scalar1=mvs[:, 0, t: t + 1],
                        scalar2=rstds[:, t: t + 1],
                        op0=mybir.AluOpType.subtract,
                        op1=mybir.AluOpType.mult,
                    )
                    tp = ps_proj.tile([128, C], BF16, name="ps_tp", tag="proj")
                    for c in range(C_CHUNKS):
                        nc.tensor.transpose(
                            tp[:, c * 128: (c + 1) * 128],
                            z_t[:, c * 128: (c + 1) * 128],
                            ident_s,
                        )
                    # zT[w][:, c, t*128:(t+1)*128] <- tp[:, c*128:...] as i32
                    nc.vector.tensor_copy(
                        zT[w][:, :, t * 128: (t + 1) * 128].bitcast(I32),
                        tp[:].bitcast(I32).rearrange(
                            "p (c n) -> p c n", c=C_CHUNKS),
                    )

                H2 = TOK_TILES // 2
                ths = []
                for t in range(H2):
                    ths.append(lambda t=t: th_a(t))
                ths.append(lambda: th_b(0, H2))
                for t in range(H2):
                    ths.append(lambda t=t: th_c(t))
                for t in range(H2, TOK_TILES):
                    ths.append(lambda t=t: th_a(t))
                ths.append(lambda: th_b(H2, TOK_TILES))
                for t in range(H2, TOK_TILES):
                    ths.append(lambda t=t: th_c(t))
                if split_dma:
                    dmas = [(lambda t=t: th_dma(t)) for t in range(TOK_TILES)]
                    return dmas, ths
                return ths

            # ---- QKV projections (bf16, 4-step c accumulation) -------
            def qkv_thunks(w):
                ths = []
                for name, w_s, b_s in (("q", wq_s, bq_s), ("k", wk_s, bk_s)):
                    for m in range(M_GROUPS):
                        for n in range(Q_TILES):
                            def th(name=name, w_s=w_s, b_s=b_s, m=m, n=n):
                                dst = qkt[(w, name, m)]
                                ps = ps_proj.tile(
                                    [128, 512], F32, name="ps_p", tag="proj"
                                )
                                if qkv_dr:
                                    for c2 in range(C_CHUNKS // 2):
                                        nc.tensor.matmul(
                                            ps,
                                            lhsT=w_s[:, 2 * c2: 2 * c2 + 2,
                                                     m * 128: (m + 1) * 128],
                                            rhs=zT[w][:, 2 * c2: 2 * c2 + 2,
                                                      n * 512: (n + 1) * 512],
                                            start=(c2 == 0),
                                            stop=(c2 == C_CHUNKS // 2 - 1),
                                            perf_mode=(
                                                mybir.MatmulPerfMode.DoubleRow
                                            ),
                                        )
                                    nc.vector.tensor_scalar(
                                        out=dst[:, n * 512: (n + 1) * 512],
                                        in0=ps,
                                        scalar1=float(1.0 / WSCALE),
                                        scalar2=b_s[:, m: m + 1],
                                        op0=mybir.AluOpType.mult,
                                        op1=mybir.AluOpType.add,
                                    )
                                else:
                                    for c in range(C_CHUNKS):
                                        nc.tensor.matmul(
                                            ps,
                                            lhsT=w_s[:, c,
                                                     m * 128: (m + 1) * 128],
                                            rhs=zT[w][:, c,
                                                      n * 512: (n + 1) * 512],
                                            start=(c == 0),
                                            stop=(c == C_CHUNKS - 1),
                                        )
                                    nc.vector.tensor_scalar_add(
                                        out=dst[:, n * 512: (n + 1) * 512],
                                        in0=ps,
                                        scalar1=b_s[:, m: m + 1],
                                    )
                            ths.append(th)
                for t in range(TOK_TILES):
                    def th(t=t):
                        ps = ps_proj.tile([128, 512], F32, name="ps_p",
                                          tag="proj")
                        psv = ps[:, :HK]
                        if qkv_dr:
                            for c2 in range(C_CHUNKS // 2):
                                nc.tensor.matmul(
                                    psv,
                                    lhsT=zT[w][:, 2 * c2: 2 * c2 + 2,
                                               t * 128: (t + 1) * 128],
                                    rhs=wv_s[:, 2 * c2: 2 * c2 + 2, :],
                                    start=(c2 == 0),
                                    stop=(c2 == C_CHUNKS // 2 - 1),
                                    perf_mode=mybir.MatmulPerfMode.DoubleRow,
                                )
                            nc.vector.tensor_scalar(
                                out=vs[w][:, t, :, :, 0, :],
                                in0=psv.rearrange(
                                    "p (u j k) -> p u j k", u=4, j=2),
                                scalar1=float(1.0 / WSCALE),
                                scalar2=0.0,
                                op0=mybir.AluOpType.mult,
                                op1=mybir.AluOpType.add,
                            )
                        else:
                            for c in range(C_CHUNKS):
                                nc.tensor.matmul(
                                    psv,
                                    lhsT=zT[w][:, c, t * 128: (t + 1) * 128],
                                    rhs=wv_s[:, c, :],
                                    start=(c == 0),
                                    stop=(c == C_CHUNKS - 1),
                                )
                            nc.vector.tensor_copy(
                                vs[w][:, t, :, :, 0, :],
                                psv.rearrange(
                                    "p (u j k) -> p u j k", u=4, j=2),
                            )
                    ths.append(th)
                return ths

            # ---- output projection + residual ------------------------
            def outproj_thunk(w, t):
                def th():
                    oT = oTs[w]
                    ps = ps_proj.tile([128, 512], F32, name="ps_p", tag="proj")
                    if fp8_out:
                        nc.tensor.matmul(
                            ps,
                            lhsT=oT[:, :, t * 128: (t + 1) * 128],
                            rhs=wo_s,
                            perf_mode=mybir.MatmulPerfMode.DoubleRow,
                        )
                    else:
                        for g in range(2):
                            nc.tensor.matmul(
                                ps,
                                lhsT=oT[:, g, t * 128: (t + 1) * 128],
                                rhs=wo_s[:, g, :],
                                start=(g == 0),
                                stop=(g == 1),
                            )
                    o_t = outp.tile([128, C], F32, tag="o")
                    sc = (1.0 / (OSCALE * OSCALE)) if fp8_out else 1.0
                    nc.vector.affine_then_add(
                        out=o_t, in0=ps, in1=xs[w][t], scale=sc, bias=0.0
                    )
                    if has_bo:
                        nc.vector.tensor_add(out=o_t, in0=o_t, in1=bo_s)
                    nc.sync.dma_start(
                        out_d[(w * TOK_TILES + t) * 128:][:128, :], o_t
                    )
                return th

            # ---- attention window: (qt, m) passes of 8 s-chunk steps -
            def attn_emit(w, inject):
                """Emit N_STEPS steps; inject[step] thunks are emitted
                between the scores quad and the (lagged) AV of each step."""
                oT = oTs[w]
                if not do_attn:
                    nc.gpsimd.memset(oT, 0.001)
                    for i in sorted(inject):
                        for th in inject[i]:
                            th()
                    return
                passes = [(qt, m) for qt in range(Q_TILES)
                          for m in range(M_GROUPS)]
                step = 0
                exi = [0]

                def emit_quad(qt, m, cch):
                    for g in range(4):
                        tgt = scA if g < 2 else scB
                        nc.tensor.matmul(
                            tgt[:, (g % 2) * 512: (g % 2 + 1) * 512],
                            lhsT=qkt[(w, "k", m)][
                                g * 32: (g + 1) * 32,
                                cch * 128: (cch + 1) * 128,
                            ],
                            rhs=qkt[(w, "q", m)][
                                g * 32: (g + 1) * 32,
                                qt * 512: (qt + 1) * 512,
                            ],
                            tile_position=(g * 32, 0),
                            skip_group_check=True,
                        )

                def emit_exps():
                    """exp scA -> exa, scB -> exb (engine per schedule)."""
                    exa = ex.tile([128, 1024], BF16, name="exa", tag="exp")
                    exb = ex.tile([128, 1024], BF16, name="exb", tag="exp")
                    for src, dst in ((scA, exa), (scB, exb)):
                        exi[0] += 1
                        if dve_exp_mod and exi[0] % dve_exp_mod == 0:
                            nc.vector.tensor_scalar(
                                out=dst[:].bitcast(I16), in0=src,
                                scalar1=float(
                                    1.4426950408889634 * 128.0 * SCALE),
                                scalar2=float(127.0 * 128.0 - 5.5),
                                op0=mybir.AluOpType.mult,
                                op1=mybir.AluOpType.add,
                            )
                        else:
                            nc.scalar.activation(
                                out=dst, in_=src,
                                func=mybir.ActivationFunctionType.Exp,
                                scale=float(SCALE),
                            )
                    return exa, exb

                def emit_av(m, cch, exa, exb, accs):
                    for u_loc, ext in ((0, exa), (1, exb)):
                        u = 2 * m + u_loc
                        acc = accs[u_loc]
                        for j in range(2):
                            nc.tensor.matmul(
                                acc[j * 64: (j + 1) * 64, :],
                                lhsT=vs[w][:, cch, u, j, :, :],
                                rhs=ext[:, j * 512: (j + 1) * 512],
                                start=(cch == 0),
                                stop=(cch == S_CHUNKS - 1),
                                tile_position=(0, j * 64),
                                skip_group_check=True,
                            )

                def emit_norm(qt, m, accs):
                    # acc rows: [num_a | den_a | num_b | den_b]
                    for u_loc in range(2):
                        acc = accs[u_loc]
                        rec = tmp.tile([64, 512], F32, tag="rec")
                        if recip_mode == "slow":
                            nc.vector.reciprocal(
                                out=rec[0:32, :], in_=acc[32:64, :])
                            nc.vector.reciprocal(
                                out=rec[32:64, :], in_=acc[96:128, :])
                        elif recip_mode == "copy_fast":
                            dcp = tmp.tile([64, 512], F32, tag="dcp")
                            nc.scalar.activation(
                                out=dcp[0:32, :], in_=acc[32:64, :],
                                func=mybir.ActivationFunctionType.Copy)
                            nc.scalar.activation(
                                out=dcp[32:64, :], in_=acc[96:128, :],
                                func=mybir.ActivationFunctionType.Copy)
                            nc.vector.reciprocal_approx_fast(
                                out=rec, in_=dcp)
                        else:
                            nc.vector.reciprocal_approx_fast(
                                out=rec[0:32, :], in_=acc[32:64, :])
                            nc.vector.reciprocal_approx_fast(
                                out=rec[32:64, :], in_=acc[96:128, :])
                        for j in range(2):
                            nc.vector.tensor_mul(
                                out=oT[u_loc * 64 + j * 32:
                                       u_loc * 64 + (j + 1) * 32, m,
                                       qt * 512: (qt + 1) * 512],
                                in0=acc[j * 64: j * 64 + 32, :],
                                in1=rec[j * 32: (j + 1) * 32, :],
                            )

                from collections import deque
                pend = deque()  # (qt, m, cch, exa, exb, accs)
                for (qt, m) in passes:
                    accs = None
                    for cch in range(S_CHUNKS):
                        if cch == 0:
                            accs = [
                                ps_acc.tile([128, 512], F32, name="ps_av",
                                            tag="acc")
                                for _ in range(2)
                            ]
                        emit_quad(qt, m, cch)
                        for th in inject.get(step, ()):
                            th()
                        # AV lags 1 step normally, 2-3 steps across pass
                        # boundaries so the new pass's AV (blocked on the
                        # previous norm freeing its acc bank) does not
                        # head-of-line block the quads.
                        lag = 3 if cch < 2 else 1
                        while len(pend) >= lag + 1 or (
                                pend and pend[0][2] == S_CHUNKS - 1
                                and cch >= 1):
                            pqt, pm, pc, pea, peb, pacc = pend.popleft()
                            emit_av(pm, pc, pea, peb, pacc)
                            if pc == S_CHUNKS - 1:
                                emit_norm(pqt, pm, pacc)
                        exa, exb = emit_exps()
                        pend.append((qt, m, cch, exa, exb, accs))
                        step += 1
                while pend:
                    pqt, pm, pc, pea, peb, pacc = pend.popleft()
                    emit_av(pm, pc, pea, peb, pacc)
                    if pc == S_CHUNKS - 1:
                        emit_norm(pqt, pm, pacc)
                for i in sorted(inject):
                    if i >= step:
                        for th in inject[i]:
                            th()

            # ================= schedule =============================
            lnw0 = ln_thunks(0)
            lnw1_dma, lnw1 = ln_thunks(1, split_dma=True)
            # head: x(w0) DMAs + stats first, consts, x(w1) DMAs, warm-up
            for th in lnw0[:4]:
                th()
            const_thunk()
            for th in lnw1_dma:
                th()
            for th in vs_memsets:
                th()
            if warm_mms:
                # PE warm-up: harmless ident matmuls into a scratch bank
                # (keeps the HAM activity window busy during the DVE head)
                wps = ps_proj.tile([128, 512], F32, name="ps_warm",
                                   tag="proj")
                for i in range(warm_mms):
                    nc.tensor.matmul(
                        wps[:, 0:128], lhsT=ident_s, rhs=ident_s,
                        start=True, stop=True,
                    )
            for th in lnw0[4:]:
                th()
            # QKV(w0) interleaved with LN(w1) (PE vs DVE work)
            ths_w0 = qkv_thunks(0)
            mix = []
            i0 = i1 = 0
            while i0 < len(ths_w0) or i1 < len(lnw1):
                if i0 < len(ths_w0):
                    mix.append(ths_w0[i0]); i0 += 1
                if i1 < len(lnw1):
                    mix.append(lnw1[i1]); i1 += 1
            for th in mix:
                th()

            # attention(w0) with QKV(w1) injected
            inj0 = {}
            ths_w1 = qkv_thunks(1)
            for idx, th in enumerate(ths_w1):
                inj0.setdefault(min(4 + idx, N_STEPS - 1), []).append(th)
            attn_emit(0, inj0)

            # attention(w1) with outproj(w0) early + outproj(w1,first half)
            inj1 = {}
            for idx, t in enumerate(range(TOK_TILES)):
                inj1.setdefault(min(2 + idx * 3, N_STEPS - 1), []).append(
                    outproj_thunk(0, t)
                )
            half = N_STEPS // 2
            for idx, t in enumerate(range(TOK_TILES // 2)):
                inj1.setdefault(
                    min(half + 3 + idx * 3, N_STEPS - 1), []
                ).append(outproj_thunk(1, t))
            for idx, t in enumerate(range(TOK_TILES // 2, TOK_TILES)):
                inj1.setdefault(N_STEPS + idx, []).append(outproj_thunk(1, t))
            attn_emit(1, inj1)

            if debug_outs:
                nc.sync.dma_start(
                    dbg_zt_d[0:][:128, :],
                    zT[0].rearrange("p c w -> p (c w)"))
                nc.sync.dma_start(dbg_q_d[0:][:128, :], qkt[(0, "q", 0)])
                nc.sync.dma_start(dbg_k_d[0:][:128, :], qkt[(0, "k", 0)])
                nc.sync.dma_start(
                    dbg_v_d[0:][:128, :],
                    vs[0].rearrange("p s u j o k -> p (s u j o k)"))
                ot_f = outp.tile([128, 2 * W], F32, tag="dbgot")
                nc.vector.tensor_copy(
                    ot_f, oTs[0].rearrange("p m w -> p (m w)"))
                nc.sync.dma_start(dbg_ot_d[0:][:128, :], ot_f)

            _es.close()

    nc.compile()
    return nc


_CACHE = {}


def _get_program(has_bo):
    key = ("nc", has_bo)
    if key not in _CACHE:
        _CACHE[key] = _build_program(has_bo=has_bo)
    return _CACHE[key]


def _prep_inputs(x, ln_gamma, ln_beta, Wq, bq, Wk, bk, Wv, bv, Wo, bo,
                 fp8_out=True, qkv_dr=False):
    """Host-side constant folding + sharding. Returns per-core in_maps."""
    x = np.asarray(x, np.float32)
    g = np.asarray(ln_gamma, np.float32)
    be = np.asarray(ln_beta, np.float32)
    Wq = np.asarray(Wq, np.float32).reshape(C, HK)
    Wk = np.asarray(Wk, np.float32).reshape(C, HK)
    Wv = np.asarray(Wv, np.float32).reshape(C, HK)
    Wo2 = np.asarray(Wo, np.float32).reshape(HK, C)
    bq = np.asarray(bq, np.float32).reshape(HK)
    bk = np.asarray(bk, np.float32).reshape(HK)
    bv = np.asarray(bv, np.float32).reshape(HK)
    bo = np.asarray(bo, np.float32).reshape(C)

    # Fold LN affine into projections
    Wq_e = g[:, None] * Wq
    Wk_e = g[:, None] * Wk
    Wv_e = g[:, None] * Wv
    bq_e = be @ Wq + bq
    bk_e = be @ Wk + bk
    bv_e = be @ Wv + bv
    # softmax rows sum to 1 -> bv folds into output bias
    bo_e = bv_e @ Wo2 + bo

    bf = ml_dtypes.bfloat16
    if qkv_dr:
        wqkv_h = np.ascontiguousarray(
            np.stack([Wq_e, Wk_e, Wv_e]).reshape(3, C_CHUNKS, 128, HK)
            * WSCALE
        ).astype(ml_dtypes.float8_e4m3)
    else:
        wqkv_h = np.ascontiguousarray(
            np.stack([Wq_e, Wk_e, Wv_e]).reshape(3, C_CHUNKS, 128, HK)
        ).astype(bf)
    if fp8_out:
        # [128, 2, C]: row p, (i, c) -> Wo[p + 128 i, c] * OSCALE
        wo_h = np.ascontiguousarray(
            (Wo2 * OSCALE).reshape(2, 128, C).transpose(1, 0, 2)
        ).astype(ml_dtypes.float8_e4m3)
    else:
        wo_h = Wo2.reshape(2, 128, C).astype(bf)
    bqk_h = np.ascontiguousarray(
        np.stack(
            [bq_e[0:128], bq_e[128:256], bk_e[0:128], bk_e[128:256]], axis=1
        )
    ).astype(np.float32)
    bo_h = bo_e.reshape(1, C).astype(np.float32)

    has_bo = bool(np.any(bo_e != 0))
    xw = np.ascontiguousarray(x.reshape(NW, W, C))
    in_maps = []
    for i in range(N_CORES):
        shard = np.ascontiguousarray(
            xw[i * WPC: (i + 1) * WPC].reshape(WPC * W, C)
        )
        m = {
            "x": shard,
            "wqkv": wqkv_h, "wo": wo_h, "bqk": bqk_h,
            "ident": np.eye(128, dtype=bf),
        }
        if has_bo:
            m["bo"] = bo_h
        in_maps.append(m)
    return in_maps


def kernel(x, ln_gamma, ln_beta, Wq, bq, Wk, bk, Wv, bv, Wo, bo):
    in_maps = _prep_inputs(x, ln_gamma, ln_beta, Wq, bq, Wk, bk, Wv, bv,
                           Wo, bo)
    nc = _get_program(has_bo="bo" in in_maps[0])
    res = run_bass_kernel_spmd(nc, in_maps, core_ids=list(range(N_CORES)))
    out = np.concatenate(
        [res.results[i]["out"] for i in range(N_CORES)], axis=0
    )
    return np.ascontiguousarray(out.reshape(B, T, C)).astype(np.float32)
